# revision 17
# baseline (speedup 1.0000x reference)
"""MixER MoE-hypernetwork kernel for 8 Trainium2 NeuronCores (v3).

Expert-parallel: core e handles expert e (NEXP == n_cores == 8).

v3 = v2's SBUF-resident delta + affine gathers, plus:
  - wavefront: phase-1 H-block streaming is interleaved with phase-2 layer
    emission (region order W1|W2|W3|W4), so the Silu stream starts ~15us in
    instead of after the full H load;
  - points-split: phase 2 runs twice over 1024-point halves, layer-major
    across all 16 envs, which keeps the live h-tile set at ~half SBUF cost;
  - env-pair packing: L1 runs env pairs in PE row groups (K=64 each,
    tile_position (0,0)/(64,0)), L4 in col groups (M=64 each,
    tile_position (0,0)/(0,64)) — recovers the half-idle PE array;
  - bf16 pair-packed output [8 pairs, 128, 2048], unpacked+summed on host.

delta_sb [128, 40960] bf16: partition 32*g + env, free col j*512 + c holds
H_dev col j*2048 + g*512 + c.  Free-col region layout S:
  W1 [0,4096):      S = q*256 + w,            fw1[d=g*16+q, w]
  W2 [4096,20480):  S-C2 = q*512 + kk*256+v,  fw2[p=g*32+q, kk*256+v]
  W3 [20480,36864): same as W2
  W4 [36864,40960): S-C4 = q*128 + kk*64+d,   fw4[p=g*32+q, kk*64+d]
so dw_l(env) = delta_sb[env::32, region] zipped into the [P, cols] tile.

Bias deltas come from a tiny transposed matmul with the 896 bias rows of H
(b4 duplicated into both row halves for the pair-packed L4 epilogue).
"""
import os
import numpy as np
import ml_dtypes

import concourse.bass as bass
import concourse.bacc as bacc
import concourse.tile as tile
from concourse import mybir
from concourse.bass_utils import run_bass_kernel_spmd

# ---- problem dims (hardcoded; must match the grader's setup_inputs) ----
DATA, WIDTH, CTXD, NEXP, ENVS, NPTS = 64, 256, 128, 8, 16, 2048
SIZES = [WIDTH * DATA, WIDTH, WIDTH * WIDTH, WIDTH, WIDTH * WIDTH, WIDTH,
         DATA * WIDTH, DATA]
OFFS = np.cumsum([0] + SIZES)
NET_USED = int(OFFS[-1])          # 164672

NBLK = 80
BLK = 2048
NWCOL = NBLK * 512                # 40960 free cols in delta_sb
C1, C2, C3, C4 = 0, 4096, 20480, 36864
HBPAD = 896                       # 7 chunks of 128 (b4 duplicated)
NPAIR = ENVS // 2
HNP = NPTS // 2                   # 1024-point half

F32 = mybir.dt.float32
BF16 = mybir.dt.bfloat16
BF16_NP = ml_dtypes.bfloat16

N_CORES = 8
TRACE = os.environ.get("MIXER_TRACE", "0") == "1"

if TRACE:
    # The agent image's antenv lacks axon_hooks, so run_bass_kernel_spmd's
    # trace path can't find the NTFF profile hook. Shim it with the ctypes
    # hook factory that trn_boot ships. Profiling-only; inert when TRACE=0.
    try:
        from antenv.axon_hooks import get_axon_ntff_profile_hook  # noqa: F401
    except ImportError:
        import sys as _sys
        import types as _types
        try:
            from trn_agent_boot.trn_boot import _ntff_profile_via_ctypes
            _hook = _ntff_profile_via_ctypes("/opt/axon/libaxon_pjrt.so")
            import antenv as _antenv
            _mod = _types.ModuleType("antenv.axon_hooks")
            _mod.get_axon_ntff_profile_hook = lambda: _hook
            _mod.set_axon_ntff_profile_hook = lambda h: None
            _sys.modules["antenv.axon_hooks"] = _mod
            _antenv.axon_hooks = _mod
        except Exception as _e:  # pragma: no cover - profiling is best-effort
            print(f"NTFF hook shim failed: {_e}")

LAST_RESULTS = None  # BassKernelResults of the most recent run (for test.py)

_NC_CACHE = {}
_PERM_CACHE = {}


# --------------------------------------------------------------------------
# host-side preprocessing
# --------------------------------------------------------------------------
def _build_perm():
    """perm[dev_col] = orig H row, for the 80-block weight tensor ht."""
    if "perm" in _PERM_CACHE:
        return _PERM_CACHE["perm"]
    perm = np.zeros(NBLK * BLK, dtype=np.int64)

    def dev(S, g):
        j, c = S // 512, S % 512
        return j * BLK + g * 512 + c

    # W1: dev (g, q, w): S = C1 + q*256 + w; d = g*16+q; orig = w*64 + d
    g, q, w = np.meshgrid(np.arange(4), np.arange(16), np.arange(256),
                          indexing="ij")
    perm[dev(C1 + q * 256 + w, g)] = OFFS[0] + w * DATA + g * 16 + q

    # W2/W3: dev (g, q, kk, v): S = C + q*512 + kk*256 + v;
    #        w = kk*128 + g*32 + q; orig = v*256 + w
    g, q, kk, v = np.meshgrid(np.arange(4), np.arange(32), np.arange(2),
                              np.arange(256), indexing="ij")
    wfull = kk * 128 + g * 32 + q
    perm[dev(C2 + q * 512 + kk * 256 + v, g)] = OFFS[2] + v * WIDTH + wfull
    perm[dev(C3 + q * 512 + kk * 256 + v, g)] = OFFS[4] + v * WIDTH + wfull

    # W4: dev (g, q, kk, d): S = C4 + q*128 + kk*64 + d;
    #     w = kk*128 + g*32 + q; orig = d*256 + w
    g, q, kk, d = np.meshgrid(np.arange(4), np.arange(32), np.arange(2),
                              np.arange(64), indexing="ij")
    wfull = kk * 128 + g * 32 + q
    perm[dev(C4 + q * 128 + kk * 64 + d, g)] = OFFS[6] + d * WIDTH + wfull

    bias_rows = np.concatenate([
        OFFS[1] + np.arange(WIDTH), OFFS[3] + np.arange(WIDTH),
        OFFS[5] + np.arange(WIDTH),
        OFFS[7] + np.arange(DATA), OFFS[7] + np.arange(DATA)])  # b4 twice
    _PERM_CACHE["perm"] = (perm, bias_rows)
    return perm, bias_rows


def _prep_inputs(y, ctx, W, b, H, G, beta):
    """Returns in_maps: one dict per core."""
    perm, bias_rows = _build_perm()

    # gate softmax on host (tiny)
    logits = ctx.astype(np.float32) @ G.astype(np.float32).T      # [B, E]
    m = logits.max(-1, keepdims=True)
    eg = np.exp(logits - m)
    gate = (eg / eg.sum(-1, keepdims=True)).astype(np.float32)

    # pair-stacked y: [8, 128, 2048], rows 0-63 env 2p, 64-127 env 2p+1
    yT = y.transpose(0, 2, 1)                                     # [16,64,2048]
    ytp = np.ascontiguousarray(
        yT.reshape(NPAIR, 2 * DATA, NPTS)).astype(BF16_NP)
    ctxT = np.ascontiguousarray(ctx.T).astype(BF16_NP)            # [128, 16]
    # per-group env placement skewed by 4: env e -> partition 32g + 4g + e,
    # so each env's delta rows {e+36g} hit 4 different SDMA engines
    ctxT128 = np.zeros((CTXD, 128), dtype=BF16_NP)
    for g in range(4):
        ctxT128[:, 32 * g + 4 * g:32 * g + 4 * g + ENVS] = ctxT

    in_maps = []
    for e in range(NEXP):
        be = float(beta[e])
        ib = np.float32(1.0 / be)

        scale = np.ones(NET_USED, dtype=np.float32)
        scale[OFFS[2]:OFFS[2] + WIDTH * WIDTH] = ib
        scale[OFFS[4]:OFFS[4] + WIDTH * WIDTH] = ib
        scale[OFFS[6]:OFFS[6] + DATA * WIDTH] = ib
        Hp = (H[e] * scale[:, None])[perm]                        # [163840,128]
        ht = np.ascontiguousarray(
            Hp.T.reshape(CTXD, NBLK // 2, 2 * BLK).transpose(1, 0, 2)
        ).astype(BF16_NP)

        # bias hypernet rows, transposed: [128 ctx, 896]
        bscale = np.concatenate([np.full(768, be, np.float32),
                                 np.ones(128, np.float32)])
        hb = np.ascontiguousarray(
            (H[e][bias_rows] * bscale[:, None]).T).astype(BF16_NP)

        # base biases per 128-chunk: [128, 7] f32 (chunk 6 = b4 twice)
        bb = np.zeros((128, 7), dtype=np.float32)
        bb[:, 0] = be * b[0][e][:128]; bb[:, 1] = be * b[0][e][128:]
        bb[:, 2] = be * b[1][e][:128]; bb[:, 3] = be * b[1][e][128:]
        bb[:, 4] = be * b[2][e][:128]; bb[:, 5] = be * b[2][e][128:]
        bb[:64, 6] = b[3][e]; bb[64:, 6] = b[3][e]

        w1t = W[0][e].T.astype(np.float32)                        # [64, 256]
        w1tp = np.ascontiguousarray(
            np.concatenate([w1t, w1t], axis=0)).astype(BF16_NP)   # [128, 256]
        w2t = np.ascontiguousarray(
            (W[1][e].T * ib).reshape(2, 128, WIDTH).transpose(1, 0, 2)
            .reshape(128, 2 * WIDTH)).astype(BF16_NP)             # [128, 512]
        w3t = np.ascontiguousarray(
            (W[2][e].T * ib).reshape(2, 128, WIDTH).transpose(1, 0, 2)
            .reshape(128, 2 * WIDTH)).astype(BF16_NP)
        w4t = (W[3][e].T * ib).reshape(2, 128, DATA).transpose(1, 0, 2) \
            .reshape(128, 2 * DATA)
        w4tp = np.ascontiguousarray(
            np.concatenate([w4t, w4t], axis=1)).astype(BF16_NP)   # [128, 256]

        # per-pair gate, row-half-selected: gatep[p, pr] = gate[2pr + (p>=64)]
        gatep = np.zeros((128, NPAIR), dtype=np.float32)
        gatep[:64, :] = gate[::2, e]
        gatep[64:, :] = gate[1::2, e]

        in_maps.append({
            "ht": ht, "hb": hb, "bb": bb, "ctxt": ctxT128, "ytp": ytp,
            "w1tp": w1tp, "w2t": w2t, "w3t": w3t, "w4tp": w4tp,
            "gatep": gatep,
            "beta": np.array([be], dtype=np.float32),
        })
    return in_maps


# --------------------------------------------------------------------------
# device kernel (SPMD program, one expert per core)
# --------------------------------------------------------------------------
def _build_nc():
    if "nc" in _NC_CACHE:
        return _NC_CACHE["nc"]
    nc = bacc.Bacc()
    P = 128

    ht = nc.declare_dram_parameter("ht", [NBLK // 2, CTXD, 2 * BLK], BF16, isOutput=False)
    hb = nc.declare_dram_parameter("hb", [CTXD, HBPAD], BF16, isOutput=False)
    bb = nc.declare_dram_parameter("bb", [P, 7], F32, isOutput=False)
    ctxt = nc.declare_dram_parameter("ctxt", [CTXD, 128], BF16, isOutput=False)
    ytp = nc.declare_dram_parameter("ytp", [NPAIR, P, NPTS], BF16, isOutput=False)
    w1tp = nc.declare_dram_parameter("w1tp", [P, WIDTH], BF16, isOutput=False)
    w2t = nc.declare_dram_parameter("w2t", [P, 2 * WIDTH], BF16, isOutput=False)
    w3t = nc.declare_dram_parameter("w3t", [P, 2 * WIDTH], BF16, isOutput=False)
    w4tp = nc.declare_dram_parameter("w4tp", [P, 4 * DATA], BF16, isOutput=False)
    gatep = nc.declare_dram_parameter("gatep", [P, NPAIR], F32, isOutput=False)
    beta = nc.declare_dram_parameter("beta", [1], F32, isOutput=False)
    out = nc.declare_dram_parameter("out", [NPAIR, P, NPTS], BF16, isOutput=True)

    SILU = mybir.ActivationFunctionType.Silu
    MULT, ADD = mybir.AluOpType.mult, mybir.AluOpType.add

    def _bcast(handle, parts):
        ap = handle[:]
        return bass.AP(tensor=ap.tensor, offset=ap.offset,
                       ap=[[0, parts]] + list(ap.ap))

    with tile.TileContext(nc) as tc:
        with tc.tile_pool(name="const", bufs=1) as const, \
             tc.tile_pool(name="delta", bufs=1) as dpool, \
             tc.tile_pool(name="p1ps", bufs=2, space="PSUM") as p1ps, \
             tc.tile_pool(name="htp", bufs=3) as htp, \
             tc.tile_pool(name="fw", bufs=2) as fwp, \
             tc.tile_pool(name="fw3", bufs=3) as fwp3, \
             tc.tile_pool(name="ypool", bufs=2) as ypool, \
             tc.tile_pool(name="hpool", bufs=36) as hpool, \
             tc.tile_pool(name="opool", bufs=3) as opool, \
             tc.tile_pool(name="psp", bufs=3, space="PSUM") as psp:

            delta_sb = dpool.tile([P, NWCOL], BF16)

            # ---- minimal consts needed by phase 1 ----
            ctx_sb = const.tile([CTXD, 128], BF16)
            nc.sync.dma_start(out=ctx_sb, in_=ctxt[:, :])
            beta_sb = const.tile([P, 1], F32)
            gatep_sb = const.tile([P, NPAIR], F32)
            hb_sb = const.tile([CTXD, HBPAD], BF16)
            bb_sb = const.tile([P, 7], F32)
            w1tp_sb = const.tile([P, WIDTH], BF16)
            w2t_sb = const.tile([P, 2 * WIDTH], BF16)
            w3t_sb = const.tile([P, 2 * WIDTH], BF16)
            w4tp_sb = const.tile([P, 4 * DATA], BF16)
            fb_all = const.tile([P, 7 * ENVS], F32)
            fb4gp = const.tile([P, NPAIR], F32)

            def emit_consts_and_bias():
                nc.sync.dma_start(out=beta_sb, in_=_bcast(beta, P))
                nc.sync.dma_start(out=gatep_sb, in_=gatep[:, :])
                nc.sync.dma_start(out=hb_sb, in_=hb[:, :])
                nc.sync.dma_start(out=bb_sb, in_=bb[:, :])
                nc.sync.dma_start(out=w1tp_sb, in_=w1tp[:, :])
                nc.sync.dma_start(out=w2t_sb, in_=w2t[:, :])
                nc.sync.dma_start(out=w3t_sb, in_=w3t[:, :])
                nc.sync.dma_start(out=w4tp_sb, in_=w4tp[:, :])
                for k in range(7):
                    psb = p1ps.tile([P, 512], F32, tag="p1", name=f"psb_{k}")
                    nc.tensor.matmul(
                        psb[:, 0:ENVS],
                        lhsT=hb_sb[:, k * 128:(k + 1) * 128],
                        rhs=ctx_sb[:, 0:ENVS], start=True, stop=True)
                    nc.vector.tensor_scalar(
                        out=fb_all[:, k * ENVS:(k + 1) * ENVS],
                        in0=psb[:, 0:ENVS],
                        scalar1=bb_sb[:, k:k + 1], scalar2=None, op0=ADD)
                nc.vector.tensor_mul(
                    out=fb4gp[0:DATA, :],
                    in0=fb_all[0:DATA, 6 * ENVS + 0:7 * ENVS:2],
                    in1=gatep_sb[0:DATA, :])
                nc.vector.tensor_mul(
                    out=fb4gp[DATA:P, :],
                    in0=fb_all[DATA:P, 6 * ENVS + 1:7 * ENVS:2],
                    in1=gatep_sb[DATA:P, :])

            # ---- emission helpers ----
            def emit_block(jj):
                # one 1MB DMA covering blocks 2*jj and 2*jj+1
                htt = htp.tile([CTXD, 2 * BLK], BF16, tag="ht", name=f"ht_{jj}")
                nc.sync.dma_start(out=htt, in_=ht[jj])
                for sub in range(2):
                    j = 2 * jj + sub
                    ps = p1ps.tile([P, 512], F32, tag="p1", name=f"ps1_{j}")
                    for g in range(4):
                        nc.tensor.matmul(
                            ps[32 * g:32 * (g + 1), :],
                            lhsT=ctx_sb[:, 32 * g:32 * (g + 1)],
                            rhs=htt[:, sub * BLK + g * 512:
                                    sub * BLK + (g + 1) * 512],
                            start=True, stop=True,
                            tile_position=(0, 32 * g))
                    nc.vector.tensor_copy(
                        out=delta_sb[:, j * 512:(j + 1) * 512], in_=ps)

            hmap = {}   # (pr, half) -> {env_slot: [h_mt0, h_mt1]}
            fw1map, fw23map, fw4map, ymap = {}, {}, {}, {}

            def prep_L1(pr, half):
                eA, eB = 2 * pr, 2 * pr + 1
                dw1 = fwp.tile([P, WIDTH], BF16, tag="dw1",
                               name=f"dw1_{pr}_{half}")
                nc.gpsimd.dma_start(out=dw1[0:DATA, :],
                                    in_=delta_sb[eA::36, C1:C2])
                nc.gpsimd.dma_start(out=dw1[DATA:P, :],
                                    in_=delta_sb[eB::36, C1:C2])
                f1 = fwp.tile([P, WIDTH], BF16, tag="fw1",
                              name=f"fw1_{pr}_{half}")
                nc.vector.tensor_add(out=f1, in0=w1tp_sb, in1=dw1)
                yst = ypool.tile([P, HNP], BF16, tag="y",
                                 name=f"y_{pr}_{half}")
                yq = nc.scalar if half == 0 else nc.sync
                yq.dma_start(
                    out=yst, in_=ytp[pr][:, half * HNP:(half + 1) * HNP])
                fw1map[(pr, half)] = f1
                ymap[(pr, half)] = yst

            def mm_L1(pr, half):
                eA, eB = 2 * pr, 2 * pr + 1
                f1 = fw1map.pop((pr, half))
                yst = ymap.pop((pr, half))
                hA, hB = [], []
                for mt in range(2):
                    psA = psp.tile([P, HNP], F32, tag="ps",
                                   name=f"psA1_{pr}_{half}_{mt}")
                    psB = psp.tile([P, HNP], F32, tag="ps",
                                   name=f"psB1_{pr}_{half}_{mt}")
                    for t in range(2):
                        nc.tensor.matmul(
                            psA[:, t * 512:(t + 1) * 512],
                            lhsT=f1[0:DATA, mt * P:(mt + 1) * P],
                            rhs=yst[0:DATA, t * 512:(t + 1) * 512],
                            start=True, stop=True, tile_position=(0, 0))
                        nc.tensor.matmul(
                            psB[:, t * 512:(t + 1) * 512],
                            lhsT=f1[DATA:P, mt * P:(mt + 1) * P],
                            rhs=yst[DATA:P, t * 512:(t + 1) * 512],
                            start=True, stop=True, tile_position=(64, 0))
                    ha = hpool.tile([P, HNP], BF16, tag="h",
                                    name=f"hA1_{pr}_{half}_{mt}")
                    nc.scalar.activation(
                        out=ha, in_=psA[:, :], func=SILU,
                        bias=fb_all[:, mt * ENVS + eA:mt * ENVS + eA + 1],
                        scale=beta_sb[:, 0:1])
                    hb_ = hpool.tile([P, HNP], BF16, tag="h",
                                     name=f"hB1_{pr}_{half}_{mt}")
                    nc.scalar.activation(
                        out=hb_, in_=psB[:, :], func=SILU,
                        bias=fb_all[:, mt * ENVS + eB:mt * ENVS + eB + 1],
                        scale=beta_sb[:, 0:1])
                    hA.append(ha)
                    hB.append(hb_)
                hmap[(pr, half)] = {0: hA, 1: hB}

            def prep_L23(pr, half, li):
                base, wt = ((C2, w2t_sb), (C3, w3t_sb))[li]
                fls = []
                for slot in range(2):
                    env = 2 * pr + slot
                    dw = fwp3.tile([P, 2 * WIDTH], BF16, tag=f"dw2{slot}",
                                  name=f"dw{li}_{pr}_{half}_{slot}")
                    nc.gpsimd.dma_start(out=dw,
                                        in_=delta_sb[env::36, base:base + 16384])
                    fl = fwp3.tile([P, 2 * WIDTH], BF16, tag=f"fw2{slot}",
                                  name=f"fwl{li}_{pr}_{half}_{slot}")
                    nc.vector.tensor_add(out=fl, in0=wt, in1=dw)
                    fls.append(fl)
                fw23map[(pr, half, li)] = fls

            def mm_L23(pr, half, li):
                hprev = hmap[(pr, half)]
                fls = fw23map.pop((pr, half, li))
                hcur = {}
                for slot in range(2):
                    env = 2 * pr + slot
                    fl = fls[slot]
                    hs = []
                    for mm in range(2):
                        ps = psp.tile([P, HNP], F32, tag="ps",
                                      name=f"ps{li}_{pr}_{half}_{slot}_{mm}")
                        for kk in range(2):
                            for t in range(2):
                                nc.tensor.matmul(
                                    ps[:, t * 512:(t + 1) * 512],
                                    lhsT=fl[:, kk * WIDTH + mm * P:
                                            kk * WIDTH + (mm + 1) * P],
                                    rhs=hprev[slot][kk][:, t * 512:(t + 1) * 512],
                                    start=(kk == 0), stop=(kk == 1))
                        hn = hpool.tile([P, HNP], BF16, tag="h",
                                        name=f"h{li}_{pr}_{half}_{slot}_{mm}")
                        nc.scalar.activation(
                            out=hn, in_=ps[:, :], func=SILU,
                            bias=fb_all[:, (2 + 2 * li + mm) * ENVS + env:
                                        (2 + 2 * li + mm) * ENVS + env + 1],
                            scale=beta_sb[:, 0:1])
                        hs.append(hn)
                    hcur[slot] = hs
                hmap[(pr, half)] = hcur

            def prep_L4(pr, half):
                eA, eB = 2 * pr, 2 * pr + 1
                dw4 = fwp.tile([P, 4 * DATA], BF16, tag="dw4",
                               name=f"dw4_{pr}_{half}")
                nc.gpsimd.dma_start(out=dw4[:, 0:2 * DATA],
                                    in_=delta_sb[eA::36, C4:NWCOL])
                nc.gpsimd.dma_start(out=dw4[:, 2 * DATA:4 * DATA],
                                    in_=delta_sb[eB::36, C4:NWCOL])
                f4 = fwp.tile([P, 4 * DATA], BF16, tag="fw4",
                              name=f"fw4_{pr}_{half}")
                nc.vector.tensor_add(out=f4, in0=w4tp_sb, in1=dw4)
                fw4map[(pr, half)] = f4

            def mm_L4(pr, half):
                hprev = hmap.pop((pr, half))
                f4 = fw4map.pop((pr, half))
                ps4 = psp.tile([P, HNP], F32, tag="ps",
                               name=f"ps4_{pr}_{half}")
                for kk in range(2):
                    for t in range(2):
                        nc.tensor.matmul(
                            ps4[0:DATA, t * 512:(t + 1) * 512],
                            lhsT=f4[:, kk * DATA:(kk + 1) * DATA],
                            rhs=hprev[0][kk][:, t * 512:(t + 1) * 512],
                            start=(kk == 0), stop=(kk == 1),
                            tile_position=(0, 0))
                        nc.tensor.matmul(
                            ps4[DATA:P, t * 512:(t + 1) * 512],
                            lhsT=f4[:, 2 * DATA + kk * DATA:
                                    2 * DATA + (kk + 1) * DATA],
                            rhs=hprev[1][kk][:, t * 512:(t + 1) * 512],
                            start=(kk == 0), stop=(kk == 1),
                            tile_position=(0, 64))
                osb = opool.tile([P, HNP], BF16, tag="o",
                                 name=f"o_{pr}_{half}")
                nc.vector.tensor_scalar(
                    out=osb, in0=ps4[:, :],
                    scalar1=gatep_sb[:, pr:pr + 1],
                    scalar2=fb4gp[:, pr:pr + 1],
                    op0=MULT, op1=ADD)
                nc.sync.dma_start(
                    out=out[pr][:, half * HNP:(half + 1) * HNP], in_=osb)

            # ---- wavefront schedule (super-blocks of 2) ----
            emit_block(0)
            emit_block(1)
            emit_consts_and_bias()
            emit_block(2)
            emit_block(3)
            prep_L1(0, 0)
            prep_L1(1, 0)
            for pr in range(NPAIR):                     # L1 h0 | W2 stream
                mm_L1(pr, 0)
                emit_block(4 + 2 * pr)
                emit_block(5 + 2 * pr)
                if pr + 2 < NPAIR:
                    prep_L1(pr + 2, 0)
            prep_L23(0, 0, 0)
            prep_L23(1, 0, 0)
            for pr in range(NPAIR):                     # L2 h0 | W3 stream
                mm_L23(pr, 0, 0)
                emit_block(20 + 2 * pr)
                emit_block(21 + 2 * pr)
                if pr + 2 < NPAIR:
                    prep_L23(pr + 2, 0, 0)
            prep_L23(0, 0, 1)
            prep_L23(1, 0, 1)
            for pr in range(NPAIR):                     # L3 h0 | W4 stream
                mm_L23(pr, 0, 1)
                if pr < 4:
                    emit_block(36 + pr)
                if pr + 2 < NPAIR:
                    prep_L23(pr + 2, 0, 1)
                else:
                    prep_L1(pr - 6, 1)
            prep_L4(0, 0)
            prep_L4(1, 0)
            for pr in range(NPAIR):                     # seam: keep ACT fed
                mm_L1(pr, 1)
                mm_L4(pr, 0)
                if pr + 2 < NPAIR:
                    prep_L1(pr + 2, 1)
                    prep_L4(pr + 2, 0)
                else:
                    prep_L23(pr - 6, 1, 0)
            for pr in range(NPAIR):
                mm_L23(pr, 1, 0)
                if pr + 2 < NPAIR:
                    prep_L23(pr + 2, 1, 0)
                else:
                    prep_L23(pr - 6, 1, 1)
            for pr in range(NPAIR):
                mm_L23(pr, 1, 1)
                if pr + 2 < NPAIR:
                    prep_L23(pr + 2, 1, 1)
                else:
                    prep_L4(pr - 6, 1)
            for pr in range(NPAIR):
                mm_L4(pr, 1)
                if pr + 2 < NPAIR:
                    prep_L4(pr + 2, 1)

    nc.compile()
    _NC_CACHE["nc"] = nc
    return nc


# --------------------------------------------------------------------------
# entry point
# --------------------------------------------------------------------------
def kernel(t, y, ctx, W1, b1, W2, b2, W3, b3, W4, b4, H, G, beta):
    global LAST_RESULTS
    y = np.asarray(y, np.float32)
    ctx = np.asarray(ctx, np.float32)
    H = np.asarray(H, np.float32)
    G = np.asarray(G, np.float32)
    beta = np.asarray(beta, np.float32)
    W = [np.asarray(w, np.float32) for w in (W1, W2, W3, W4)]
    b = [np.asarray(x, np.float32) for x in (b1, b2, b3, b4)]

    in_maps = _prep_inputs(y, ctx, W, b, H, G, beta)
    nc = _build_nc()
    res = run_bass_kernel_spmd(
        nc, in_maps, list(range(N_CORES)),
        trace=TRACE, trace_cores=None)
    LAST_RESULTS = res

    total = np.zeros((ENVS, DATA, NPTS), np.float32)
    for e in range(N_CORES):
        o = res.results[e]["out"].astype(np.float32)   # [8, 128, 2048]
        total[0::2] += o[:, :DATA, :]
        total[1::2] += o[:, DATA:, :]
    return np.ascontiguousarray(total.transpose(0, 2, 1))


def measure_exec_ns(inputs, iters=64, warmup=4):
    """Steady-state per-execution time of the compiled NEFF on 8 cores.

    Keeps inputs device-resident and measures the marginal wall time of
    pipelined executions. The result still contains per-call dispatch
    overhead (compare against a trivial kernel's floor for the difference).
    Used by test.py only; the grading path never calls this.
    """
    import time
    import jax
    from jax.sharding import Mesh, PartitionSpec, NamedSharding
    from jax.experimental.shard_map import shard_map
    from concourse import bass2jax, mybir as _mybir

    y = np.asarray(inputs["y"], np.float32)
    ctx = np.asarray(inputs["ctx"], np.float32)
    H = np.asarray(inputs["H"], np.float32)
    G = np.asarray(inputs["G"], np.float32)
    beta = np.asarray(inputs["beta"], np.float32)
    W = [np.asarray(inputs[k], np.float32) for k in ("W1", "W2", "W3", "W4")]
    b = [np.asarray(inputs[k], np.float32) for k in ("b1", "b2", "b3", "b4")]
    in_maps = _prep_inputs(y, ctx, W, b, H, G, beta)
    nc = _build_nc()

    bass2jax.install_neuronx_cc_hook()
    partition_name = nc.partition_id_tensor.name if nc.partition_id_tensor else None
    in_names, out_names, out_avals, zero_outs = [], [], [], []
    for alloc in nc.m.functions[0].allocations:
        if not isinstance(alloc, _mybir.MemoryLocationSet):
            continue
        name = alloc.memorylocations[0].name
        if alloc.kind == "ExternalInput":
            if name != partition_name:
                in_names.append(name)
        elif alloc.kind == "ExternalOutput":
            shape = tuple(alloc.tensor_shape)
            dtype = _mybir.dt.np(alloc.dtype)
            out_names.append(name)
            out_avals.append(jax.core.ShapedArray(shape, dtype))
            zero_outs.append(np.zeros(shape, dtype))
    n_params = len(in_names)
    all_in_names = in_names + out_names
    if partition_name is not None:
        all_in_names.append(partition_name)

    def _body(*args):
        operands = list(args)
        if partition_name is not None:
            operands.append(bass2jax.partition_id_tensor())
        outs = bass2jax._bass_exec_p.bind(
            *operands,
            out_avals=tuple(out_avals),
            in_names=tuple(all_in_names),
            out_names=tuple(out_names),
            lowering_input_output_aliases=(),
            sim_require_finite=True,
            sim_require_nnan=True,
            nc=nc,
        )
        return tuple(outs)

    devices = jax.devices()[:N_CORES]
    mesh = Mesh(np.asarray(devices), ("core",))
    nspec = NamedSharding(mesh, PartitionSpec("core"))
    n_all = n_params + len(out_names)
    sharded = jax.jit(
        shard_map(_body, mesh=mesh,
                  in_specs=(PartitionSpec("core"),) * n_all,
                  out_specs=(PartitionSpec("core"),) * len(out_names),
                  check_rep=False),
        keep_unused=True)

    concat_in = [
        np.concatenate([np.asarray(in_maps[c][k]) for c in range(N_CORES)], axis=0)
        for k in in_names
    ] + [np.zeros((N_CORES * z.shape[0], *z.shape[1:]), z.dtype) for z in zero_outs]
    dev_in = [jax.device_put(a, nspec) for a in concat_in]

    for _ in range(warmup):
        outs = sharded(*dev_in)
    jax.block_until_ready(outs)

    t0 = time.perf_counter()
    for _ in range(iters):
        outs = sharded(*dev_in)
    jax.block_until_ready(outs)
    t1 = time.perf_counter()
    per_call = (t1 - t0) / iters

    return {"pipelined_ns": per_call * 1e9}


if __name__ == "__main__":
    _build_nc()
    print("IR build OK")


# revision 18
# speedup vs baseline: 1.0906x; 1.0906x over previous
"""MixER MoE-hypernetwork kernel for 8 Trainium2 NeuronCores (v3).

Expert-parallel: core e handles expert e (NEXP == n_cores == 8).

v3 = v2's SBUF-resident delta + affine gathers, plus:
  - wavefront: phase-1 H-block streaming is interleaved with phase-2 layer
    emission (region order W1|W2|W3|W4), so the Silu stream starts ~15us in
    instead of after the full H load;
  - points-split: phase 2 runs twice over 1024-point halves, layer-major
    across all 16 envs, which keeps the live h-tile set at ~half SBUF cost;
  - env-pair packing: L1 runs env pairs in PE row groups (K=64 each,
    tile_position (0,0)/(64,0)), L4 in col groups (M=64 each,
    tile_position (0,0)/(0,64)) — recovers the half-idle PE array;
  - bf16 pair-packed output [8 pairs, 128, 2048], unpacked+summed on host.

delta_sb [128, 40960] bf16: partition 32*g + env, free col j*512 + c holds
H_dev col j*2048 + g*512 + c.  Free-col region layout S:
  W1 [0,4096):      S = q*256 + w,            fw1[d=g*16+q, w]
  W2 [4096,20480):  S-C2 = q*512 + kk*256+v,  fw2[p=g*32+q, kk*256+v]
  W3 [20480,36864): same as W2
  W4 [36864,40960): S-C4 = q*128 + kk*64+d,   fw4[p=g*32+q, kk*64+d]
so dw_l(env) = delta_sb[env::32, region] zipped into the [P, cols] tile.

Bias deltas come from a tiny transposed matmul with the 896 bias rows of H
(b4 duplicated into both row halves for the pair-packed L4 epilogue).
"""
import os
import numpy as np
import ml_dtypes

import concourse.bass as bass
import concourse.bacc as bacc
import concourse.tile as tile
from concourse import mybir
from concourse.bass_utils import run_bass_kernel_spmd

# ---- problem dims (hardcoded; must match the grader's setup_inputs) ----
DATA, WIDTH, CTXD, NEXP, ENVS, NPTS = 64, 256, 128, 8, 16, 2048
SIZES = [WIDTH * DATA, WIDTH, WIDTH * WIDTH, WIDTH, WIDTH * WIDTH, WIDTH,
         DATA * WIDTH, DATA]
OFFS = np.cumsum([0] + SIZES)
NET_USED = int(OFFS[-1])          # 164672

NBLK = 80
BLK = 2048
NWCOL = NBLK * 512                # 40960 free cols in delta_sb
C1, C2, C3, C4 = 0, 4096, 20480, 36864
HBPAD = 896                       # 7 chunks of 128 (b4 duplicated)
NPAIR = ENVS // 2
HNP = NPTS // 2                   # 1024-point half

F32 = mybir.dt.float32
BF16 = mybir.dt.bfloat16
BF16_NP = ml_dtypes.bfloat16

N_CORES = 8
TRACE = os.environ.get("MIXER_TRACE", "0") == "1"

if TRACE:
    # The agent image's antenv lacks axon_hooks, so run_bass_kernel_spmd's
    # trace path can't find the NTFF profile hook. Shim it with the ctypes
    # hook factory that trn_boot ships. Profiling-only; inert when TRACE=0.
    try:
        from antenv.axon_hooks import get_axon_ntff_profile_hook  # noqa: F401
    except ImportError:
        import sys as _sys
        import types as _types
        try:
            from trn_agent_boot.trn_boot import _ntff_profile_via_ctypes
            _hook = _ntff_profile_via_ctypes("/opt/axon/libaxon_pjrt.so")
            import antenv as _antenv
            _mod = _types.ModuleType("antenv.axon_hooks")
            _mod.get_axon_ntff_profile_hook = lambda: _hook
            _mod.set_axon_ntff_profile_hook = lambda h: None
            _sys.modules["antenv.axon_hooks"] = _mod
            _antenv.axon_hooks = _mod
        except Exception as _e:  # pragma: no cover - profiling is best-effort
            print(f"NTFF hook shim failed: {_e}")

LAST_RESULTS = None  # BassKernelResults of the most recent run (for test.py)

_NC_CACHE = {}
_PERM_CACHE = {}


# --------------------------------------------------------------------------
# host-side preprocessing
# --------------------------------------------------------------------------
def _build_perm():
    """perm[dev_col] = orig H row, for the 80-block weight tensor ht."""
    if "perm" in _PERM_CACHE:
        return _PERM_CACHE["perm"]
    perm = np.zeros(NBLK * BLK, dtype=np.int64)

    def dev(S, g):
        j, c = S // 512, S % 512
        return j * BLK + g * 512 + c

    # W1: dev (g, q, w): S = C1 + q*256 + w; d = g*16+q; orig = w*64 + d
    g, q, w = np.meshgrid(np.arange(4), np.arange(16), np.arange(256),
                          indexing="ij")
    perm[dev(C1 + q * 256 + w, g)] = OFFS[0] + w * DATA + g * 16 + q

    # W2/W3: dev (g, q, kk, v): S = C + q*512 + kk*256 + v;
    #        w = kk*128 + g*32 + q; orig = v*256 + w
    g, q, kk, v = np.meshgrid(np.arange(4), np.arange(32), np.arange(2),
                              np.arange(256), indexing="ij")
    wfull = kk * 128 + g * 32 + q
    perm[dev(C2 + q * 512 + kk * 256 + v, g)] = OFFS[2] + v * WIDTH + wfull
    perm[dev(C3 + q * 512 + kk * 256 + v, g)] = OFFS[4] + v * WIDTH + wfull

    # W4: dev (g, q, kk, d): S = C4 + q*128 + kk*64 + d;
    #     w = kk*128 + g*32 + q; orig = d*256 + w
    g, q, kk, d = np.meshgrid(np.arange(4), np.arange(32), np.arange(2),
                              np.arange(64), indexing="ij")
    wfull = kk * 128 + g * 32 + q
    perm[dev(C4 + q * 128 + kk * 64 + d, g)] = OFFS[6] + d * WIDTH + wfull

    bias_rows = np.concatenate([
        OFFS[1] + np.arange(WIDTH), OFFS[3] + np.arange(WIDTH),
        OFFS[5] + np.arange(WIDTH),
        OFFS[7] + np.arange(DATA), OFFS[7] + np.arange(DATA)])  # b4 twice
    _PERM_CACHE["perm"] = (perm, bias_rows)
    return perm, bias_rows


def _prep_inputs(y, ctx, W, b, H, G, beta):
    """Returns in_maps: one dict per core."""
    perm, bias_rows = _build_perm()

    # gate softmax on host (tiny)
    logits = ctx.astype(np.float32) @ G.astype(np.float32).T      # [B, E]
    m = logits.max(-1, keepdims=True)
    eg = np.exp(logits - m)
    gate = (eg / eg.sum(-1, keepdims=True)).astype(np.float32)

    # pair-stacked y: [8, 128, 2048], rows 0-63 env 2p, 64-127 env 2p+1
    yT = y.transpose(0, 2, 1)                                     # [16,64,2048]
    ytp = np.ascontiguousarray(
        yT.reshape(NPAIR, 2 * DATA, NPTS)).astype(BF16_NP)
    ctxT = np.ascontiguousarray(ctx.T).astype(BF16_NP)            # [128, 16]
    # per-group env placement skewed by 4: env e -> partition 32g + 4g + e,
    # so each env's delta rows {e+36g} hit 4 different SDMA engines
    ctxT128 = np.zeros((CTXD, 128), dtype=BF16_NP)
    for g in range(4):
        ctxT128[:, 32 * g + 4 * g:32 * g + 4 * g + ENVS] = ctxT

    in_maps = []
    for e in range(NEXP):
        be = float(beta[e])
        ib = np.float32(1.0 / be)

        scale = np.ones(NET_USED, dtype=np.float32)
        scale[OFFS[2]:OFFS[2] + WIDTH * WIDTH] = ib
        scale[OFFS[4]:OFFS[4] + WIDTH * WIDTH] = ib
        scale[OFFS[6]:OFFS[6] + DATA * WIDTH] = ib
        Hp = (H[e] * scale[:, None])[perm]                        # [163840,128]
        ht = np.ascontiguousarray(
            Hp.T.reshape(CTXD, NBLK // 2, 2 * BLK).transpose(1, 0, 2)
        ).astype(BF16_NP)

        # bias hypernet rows, transposed: [128 ctx, 896]
        bscale = np.concatenate([np.full(768, be, np.float32),
                                 np.ones(128, np.float32)])
        hb = np.ascontiguousarray(
            (H[e][bias_rows] * bscale[:, None]).T).astype(BF16_NP)

        # base biases per 128-chunk: [128, 7] f32 (chunk 6 = b4 twice)
        bb = np.zeros((128, 7), dtype=np.float32)
        bb[:, 0] = be * b[0][e][:128]; bb[:, 1] = be * b[0][e][128:]
        bb[:, 2] = be * b[1][e][:128]; bb[:, 3] = be * b[1][e][128:]
        bb[:, 4] = be * b[2][e][:128]; bb[:, 5] = be * b[2][e][128:]
        bb[:64, 6] = b[3][e]; bb[64:, 6] = b[3][e]

        w1t = W[0][e].T.astype(np.float32)                        # [64, 256]
        w1tp = np.ascontiguousarray(
            np.concatenate([w1t, w1t], axis=0)).astype(BF16_NP)   # [128, 256]
        w2t = np.ascontiguousarray(
            (W[1][e].T * ib).reshape(2, 128, WIDTH).transpose(1, 0, 2)
            .reshape(128, 2 * WIDTH)).astype(BF16_NP)             # [128, 512]
        w3t = np.ascontiguousarray(
            (W[2][e].T * ib).reshape(2, 128, WIDTH).transpose(1, 0, 2)
            .reshape(128, 2 * WIDTH)).astype(BF16_NP)
        w4t = (W[3][e].T * ib).reshape(2, 128, DATA).transpose(1, 0, 2) \
            .reshape(128, 2 * DATA)
        w4tp = np.ascontiguousarray(
            np.concatenate([w4t, w4t], axis=1)).astype(BF16_NP)   # [128, 256]

        # per-pair gate, row-half-selected: gatep[p, pr] = gate[2pr + (p>=64)]
        gatep = np.zeros((128, NPAIR), dtype=np.float32)
        gatep[:64, :] = gate[::2, e]
        gatep[64:, :] = gate[1::2, e]

        in_maps.append({
            "ht": ht, "hb": hb, "bb": bb, "ctxt": ctxT128, "ytp": ytp,
            "w1tp": w1tp, "w2t": w2t, "w3t": w3t, "w4tp": w4tp,
            "gatep": gatep,
            "beta": np.array([be], dtype=np.float32),
        })
    return in_maps


# --------------------------------------------------------------------------
# device kernel (SPMD program, one expert per core)
# --------------------------------------------------------------------------
def _build_nc():
    if "nc" in _NC_CACHE:
        return _NC_CACHE["nc"]
    nc = bacc.Bacc()
    P = 128

    ht = nc.declare_dram_parameter("ht", [NBLK // 2, CTXD, 2 * BLK], BF16, isOutput=False)
    hb = nc.declare_dram_parameter("hb", [CTXD, HBPAD], BF16, isOutput=False)
    bb = nc.declare_dram_parameter("bb", [P, 7], F32, isOutput=False)
    ctxt = nc.declare_dram_parameter("ctxt", [CTXD, 128], BF16, isOutput=False)
    ytp = nc.declare_dram_parameter("ytp", [NPAIR, P, NPTS], BF16, isOutput=False)
    w1tp = nc.declare_dram_parameter("w1tp", [P, WIDTH], BF16, isOutput=False)
    w2t = nc.declare_dram_parameter("w2t", [P, 2 * WIDTH], BF16, isOutput=False)
    w3t = nc.declare_dram_parameter("w3t", [P, 2 * WIDTH], BF16, isOutput=False)
    w4tp = nc.declare_dram_parameter("w4tp", [P, 4 * DATA], BF16, isOutput=False)
    gatep = nc.declare_dram_parameter("gatep", [P, NPAIR], F32, isOutput=False)
    beta = nc.declare_dram_parameter("beta", [1], F32, isOutput=False)
    out = nc.declare_dram_parameter("out", [NPAIR, P, NPTS], BF16, isOutput=True)

    SILU = mybir.ActivationFunctionType.Silu
    MULT, ADD = mybir.AluOpType.mult, mybir.AluOpType.add

    def _bcast(handle, parts):
        ap = handle[:]
        return bass.AP(tensor=ap.tensor, offset=ap.offset,
                       ap=[[0, parts]] + list(ap.ap))

    with tile.TileContext(nc) as tc:
        with tc.tile_pool(name="const", bufs=1) as const, \
             tc.tile_pool(name="delta", bufs=1) as dpool, \
             tc.tile_pool(name="p1ps", bufs=2, space="PSUM") as p1ps, \
             tc.tile_pool(name="htp", bufs=3) as htp, \
             tc.tile_pool(name="fw", bufs=2) as fwp, \
             tc.tile_pool(name="fw3", bufs=3) as fwp3, \
             tc.tile_pool(name="ypool", bufs=2) as ypool, \
             tc.tile_pool(name="hpool", bufs=36) as hpool, \
             tc.tile_pool(name="opool", bufs=3) as opool, \
             tc.tile_pool(name="psp", bufs=3, space="PSUM") as psp:

            delta_sb = dpool.tile([P, NWCOL], BF16)

            # ---- minimal consts needed by phase 1 ----
            ctx_sb = const.tile([CTXD, 128], BF16)
            nc.sync.dma_start(out=ctx_sb, in_=ctxt[:, :])
            beta_sb = const.tile([P, 1], F32)
            gatep_sb = const.tile([P, NPAIR], F32)
            hb_sb = const.tile([CTXD, HBPAD], BF16)
            bb_sb = const.tile([P, 7], F32)
            w1tp_sb = const.tile([P, WIDTH], BF16)
            w2t_sb = const.tile([P, 2 * WIDTH], BF16)
            w3t_sb = const.tile([P, 2 * WIDTH], BF16)
            w4tp_sb = const.tile([P, 4 * DATA], BF16)
            fb_all = const.tile([P, 7 * ENVS], F32)
            fb4gp = const.tile([P, NPAIR], F32)

            def emit_consts_and_bias():
                nc.sync.dma_start(out=beta_sb, in_=_bcast(beta, P))
                nc.sync.dma_start(out=gatep_sb, in_=gatep[:, :])
                nc.sync.dma_start(out=hb_sb, in_=hb[:, :])
                nc.sync.dma_start(out=bb_sb, in_=bb[:, :])
                nc.sync.dma_start(out=w1tp_sb, in_=w1tp[:, :])
                nc.sync.dma_start(out=w2t_sb, in_=w2t[:, :])
                nc.sync.dma_start(out=w3t_sb, in_=w3t[:, :])
                nc.sync.dma_start(out=w4tp_sb, in_=w4tp[:, :])
                for k in range(7):
                    psb = p1ps.tile([P, 512], F32, tag="p1", name=f"psb_{k}")
                    nc.tensor.matmul(
                        psb[:, 0:ENVS],
                        lhsT=hb_sb[:, k * 128:(k + 1) * 128],
                        rhs=ctx_sb[:, 0:ENVS], start=True, stop=True)
                    nc.vector.tensor_scalar(
                        out=fb_all[:, k * ENVS:(k + 1) * ENVS],
                        in0=psb[:, 0:ENVS],
                        scalar1=bb_sb[:, k:k + 1], scalar2=None, op0=ADD)
                nc.vector.tensor_mul(
                    out=fb4gp[0:DATA, :],
                    in0=fb_all[0:DATA, 6 * ENVS + 0:7 * ENVS:2],
                    in1=gatep_sb[0:DATA, :])
                nc.vector.tensor_mul(
                    out=fb4gp[DATA:P, :],
                    in0=fb_all[DATA:P, 6 * ENVS + 1:7 * ENVS:2],
                    in1=gatep_sb[DATA:P, :])

            # ---- emission helpers ----
            def emit_block(jj):
                # one 1MB DMA covering blocks 2*jj and 2*jj+1
                htt = htp.tile([CTXD, 2 * BLK], BF16, tag="ht", name=f"ht_{jj}")
                nc.sync.dma_start(out=htt, in_=ht[jj])
                for sub in range(2):
                    j = 2 * jj + sub
                    ps = p1ps.tile([P, 512], F32, tag="p1", name=f"ps1_{j}")
                    for g in range(4):
                        nc.tensor.matmul(
                            ps[32 * g:32 * (g + 1), :],
                            lhsT=ctx_sb[:, 32 * g:32 * (g + 1)],
                            rhs=htt[:, sub * BLK + g * 512:
                                    sub * BLK + (g + 1) * 512],
                            start=True, stop=True,
                            tile_position=(0, 32 * g))
                    nc.vector.tensor_copy(
                        out=delta_sb[:, j * 512:(j + 1) * 512], in_=ps)

            hmap = {}   # (pr, half) -> {env_slot: [h_mt0, h_mt1]}
            fw1map, fw23map, fw4map, ymap = {}, {}, {}, {}

            def prep_L1(pr, half):
                eA, eB = 2 * pr, 2 * pr + 1
                dw1 = fwp.tile([P, WIDTH], BF16, tag="dw1",
                               name=f"dw1_{pr}_{half}")
                nc.gpsimd.dma_start(out=dw1[0:DATA, :],
                                    in_=delta_sb[eA::36, C1:C2])
                nc.gpsimd.dma_start(out=dw1[DATA:P, :],
                                    in_=delta_sb[eB::36, C1:C2])
                f1 = fwp.tile([P, WIDTH], BF16, tag="fw1",
                              name=f"fw1_{pr}_{half}")
                nc.vector.tensor_add(out=f1, in0=w1tp_sb, in1=dw1)
                yst = ypool.tile([P, HNP], BF16, tag="y",
                                 name=f"y_{pr}_{half}")
                yq = nc.scalar if half == 0 else nc.sync
                yq.dma_start(
                    out=yst, in_=ytp[pr][:, half * HNP:(half + 1) * HNP])
                fw1map[(pr, half)] = f1
                ymap[(pr, half)] = yst

            def mm_L1(pr, half):
                eA, eB = 2 * pr, 2 * pr + 1
                f1 = fw1map.pop((pr, half))
                yst = ymap.pop((pr, half))
                hA, hB = [], []
                for mt in range(2):
                    psA = psp.tile([P, HNP], F32, tag="ps",
                                   name=f"psA1_{pr}_{half}_{mt}")
                    psB = psp.tile([P, HNP], F32, tag="ps",
                                   name=f"psB1_{pr}_{half}_{mt}")
                    for t in range(2):
                        nc.tensor.matmul(
                            psA[:, t * 512:(t + 1) * 512],
                            lhsT=f1[0:DATA, mt * P:(mt + 1) * P],
                            rhs=yst[0:DATA, t * 512:(t + 1) * 512],
                            start=True, stop=True, tile_position=(0, 0))
                        nc.tensor.matmul(
                            psB[:, t * 512:(t + 1) * 512],
                            lhsT=f1[DATA:P, mt * P:(mt + 1) * P],
                            rhs=yst[DATA:P, t * 512:(t + 1) * 512],
                            start=True, stop=True, tile_position=(64, 0))
                    ha = hpool.tile([P, HNP], BF16, tag="h",
                                    name=f"hA1_{pr}_{half}_{mt}")
                    nc.scalar.activation(
                        out=ha, in_=psA[:, :], func=SILU,
                        bias=fb_all[:, mt * ENVS + eA:mt * ENVS + eA + 1],
                        scale=beta_sb[:, 0:1])
                    hb_ = hpool.tile([P, HNP], BF16, tag="h",
                                     name=f"hB1_{pr}_{half}_{mt}")
                    nc.scalar.activation(
                        out=hb_, in_=psB[:, :], func=SILU,
                        bias=fb_all[:, mt * ENVS + eB:mt * ENVS + eB + 1],
                        scale=beta_sb[:, 0:1])
                    hA.append(ha)
                    hB.append(hb_)
                hmap[(pr, half)] = {0: hA, 1: hB}

            def prep_L23(pr, half, li):
                base, wt = ((C2, w2t_sb), (C3, w3t_sb))[li]
                fls = []
                for slot in range(2):
                    env = 2 * pr + slot
                    dw = fwp3.tile([P, 2 * WIDTH], BF16, tag=f"dw2{slot}",
                                  name=f"dw{li}_{pr}_{half}_{slot}")
                    nc.gpsimd.dma_start(out=dw,
                                        in_=delta_sb[env::36, base:base + 16384])
                    fl = fwp3.tile([P, 2 * WIDTH], BF16, tag=f"fw2{slot}",
                                  name=f"fwl{li}_{pr}_{half}_{slot}")
                    nc.vector.tensor_add(out=fl, in0=wt, in1=dw)
                    fls.append(fl)
                fw23map[(pr, half, li)] = fls

            def mm_L23(pr, half, li):
                hprev = hmap[(pr, half)]
                fls = fw23map.pop((pr, half, li))
                hcur = {}
                for slot in range(2):
                    env = 2 * pr + slot
                    fl = fls[slot]
                    hs = []
                    for mm in range(2):
                        ps = psp.tile([P, HNP], F32, tag="ps",
                                      name=f"ps{li}_{pr}_{half}_{slot}_{mm}")
                        for kk in range(2):
                            for t in range(2):
                                nc.tensor.matmul(
                                    ps[:, t * 512:(t + 1) * 512],
                                    lhsT=fl[:, kk * WIDTH + mm * P:
                                            kk * WIDTH + (mm + 1) * P],
                                    rhs=hprev[slot][kk][:, t * 512:(t + 1) * 512],
                                    start=(kk == 0), stop=(kk == 1))
                        hn = hpool.tile([P, HNP], BF16, tag="h",
                                        name=f"h{li}_{pr}_{half}_{slot}_{mm}")
                        nc.scalar.activation(
                            out=hn, in_=ps[:, :], func=SILU,
                            bias=fb_all[:, (2 + 2 * li + mm) * ENVS + env:
                                        (2 + 2 * li + mm) * ENVS + env + 1],
                            scale=beta_sb[:, 0:1])
                        hs.append(hn)
                    hcur[slot] = hs
                hmap[(pr, half)] = hcur

            def prep_L4(pr, half):
                eA, eB = 2 * pr, 2 * pr + 1
                dw4 = fwp.tile([P, 4 * DATA], BF16, tag="dw4",
                               name=f"dw4_{pr}_{half}")
                nc.gpsimd.dma_start(out=dw4[:, 0:2 * DATA],
                                    in_=delta_sb[eA::36, C4:NWCOL])
                nc.gpsimd.dma_start(out=dw4[:, 2 * DATA:4 * DATA],
                                    in_=delta_sb[eB::36, C4:NWCOL])
                f4 = fwp.tile([P, 4 * DATA], BF16, tag="fw4",
                              name=f"fw4_{pr}_{half}")
                nc.vector.tensor_add(out=f4, in0=w4tp_sb, in1=dw4)
                fw4map[(pr, half)] = f4

            def mm_L4(pr, half):
                hprev = hmap.pop((pr, half))
                f4 = fw4map.pop((pr, half))
                ps4 = psp.tile([P, HNP], F32, tag="ps",
                               name=f"ps4_{pr}_{half}")
                for kk in range(2):
                    for t in range(2):
                        nc.tensor.matmul(
                            ps4[0:DATA, t * 512:(t + 1) * 512],
                            lhsT=f4[:, kk * DATA:(kk + 1) * DATA],
                            rhs=hprev[0][kk][:, t * 512:(t + 1) * 512],
                            start=(kk == 0), stop=(kk == 1),
                            tile_position=(0, 0))
                        nc.tensor.matmul(
                            ps4[DATA:P, t * 512:(t + 1) * 512],
                            lhsT=f4[:, 2 * DATA + kk * DATA:
                                    2 * DATA + (kk + 1) * DATA],
                            rhs=hprev[1][kk][:, t * 512:(t + 1) * 512],
                            start=(kk == 0), stop=(kk == 1),
                            tile_position=(0, 64))
                osb = opool.tile([P, HNP], BF16, tag="o",
                                 name=f"o_{pr}_{half}")
                nc.vector.tensor_scalar(
                    out=osb, in0=ps4[:, :],
                    scalar1=gatep_sb[:, pr:pr + 1],
                    scalar2=fb4gp[:, pr:pr + 1],
                    op0=MULT, op1=ADD)
                nc.sync.dma_start(
                    out=out[pr][:, half * HNP:(half + 1) * HNP], in_=osb)

            # ---- wavefront schedule (super-blocks of 2) ----
            emit_block(0)
            emit_block(1)
            emit_consts_and_bias()
            emit_block(2)
            emit_block(3)
            for pr in range(NPAIR):                     # L1 h0 | W2 stream
                prep_L1(pr, 0)
                mm_L1(pr, 0)
                emit_block(4 + 2 * pr)
                emit_block(5 + 2 * pr)
            for pr in range(NPAIR):                     # L2 h0 | W3 stream
                prep_L23(pr, 0, 0)
                mm_L23(pr, 0, 0)
                emit_block(20 + 2 * pr)
                emit_block(21 + 2 * pr)
            for pr in range(NPAIR):                     # L3 h0 | W4 stream
                prep_L23(pr, 0, 1)
                mm_L23(pr, 0, 1)
                if pr < 4:
                    emit_block(36 + pr)
            for pr in range(NPAIR):                     # seam: keep ACT fed
                prep_L1(pr, 1)
                mm_L1(pr, 1)
                prep_L4(pr, 0)
                mm_L4(pr, 0)
            for pr in range(NPAIR):
                prep_L23(pr, 1, 0)
                mm_L23(pr, 1, 0)
            for pr in range(NPAIR):                     # tail: L4 interleaved
                prep_L23(pr, 1, 1)
                mm_L23(pr, 1, 1)
                if pr >= 1:
                    prep_L4(pr - 1, 1)
                    mm_L4(pr - 1, 1)
            prep_L4(NPAIR - 1, 1)
            mm_L4(NPAIR - 1, 1)

    nc.compile()
    _NC_CACHE["nc"] = nc
    return nc


# --------------------------------------------------------------------------
# entry point
# --------------------------------------------------------------------------
def kernel(t, y, ctx, W1, b1, W2, b2, W3, b3, W4, b4, H, G, beta):
    global LAST_RESULTS
    y = np.asarray(y, np.float32)
    ctx = np.asarray(ctx, np.float32)
    H = np.asarray(H, np.float32)
    G = np.asarray(G, np.float32)
    beta = np.asarray(beta, np.float32)
    W = [np.asarray(w, np.float32) for w in (W1, W2, W3, W4)]
    b = [np.asarray(x, np.float32) for x in (b1, b2, b3, b4)]

    in_maps = _prep_inputs(y, ctx, W, b, H, G, beta)
    nc = _build_nc()
    res = run_bass_kernel_spmd(
        nc, in_maps, list(range(N_CORES)),
        trace=TRACE, trace_cores=None)
    LAST_RESULTS = res

    total = np.zeros((ENVS, DATA, NPTS), np.float32)
    for e in range(N_CORES):
        o = res.results[e]["out"].astype(np.float32)   # [8, 128, 2048]
        total[0::2] += o[:, :DATA, :]
        total[1::2] += o[:, DATA:, :]
    return np.ascontiguousarray(total.transpose(0, 2, 1))


def measure_exec_ns(inputs, iters=64, warmup=4):
    """Steady-state per-execution time of the compiled NEFF on 8 cores.

    Keeps inputs device-resident and measures the marginal wall time of
    pipelined executions. The result still contains per-call dispatch
    overhead (compare against a trivial kernel's floor for the difference).
    Used by test.py only; the grading path never calls this.
    """
    import time
    import jax
    from jax.sharding import Mesh, PartitionSpec, NamedSharding
    from jax.experimental.shard_map import shard_map
    from concourse import bass2jax, mybir as _mybir

    y = np.asarray(inputs["y"], np.float32)
    ctx = np.asarray(inputs["ctx"], np.float32)
    H = np.asarray(inputs["H"], np.float32)
    G = np.asarray(inputs["G"], np.float32)
    beta = np.asarray(inputs["beta"], np.float32)
    W = [np.asarray(inputs[k], np.float32) for k in ("W1", "W2", "W3", "W4")]
    b = [np.asarray(inputs[k], np.float32) for k in ("b1", "b2", "b3", "b4")]
    in_maps = _prep_inputs(y, ctx, W, b, H, G, beta)
    nc = _build_nc()

    bass2jax.install_neuronx_cc_hook()
    partition_name = nc.partition_id_tensor.name if nc.partition_id_tensor else None
    in_names, out_names, out_avals, zero_outs = [], [], [], []
    for alloc in nc.m.functions[0].allocations:
        if not isinstance(alloc, _mybir.MemoryLocationSet):
            continue
        name = alloc.memorylocations[0].name
        if alloc.kind == "ExternalInput":
            if name != partition_name:
                in_names.append(name)
        elif alloc.kind == "ExternalOutput":
            shape = tuple(alloc.tensor_shape)
            dtype = _mybir.dt.np(alloc.dtype)
            out_names.append(name)
            out_avals.append(jax.core.ShapedArray(shape, dtype))
            zero_outs.append(np.zeros(shape, dtype))
    n_params = len(in_names)
    all_in_names = in_names + out_names
    if partition_name is not None:
        all_in_names.append(partition_name)

    def _body(*args):
        operands = list(args)
        if partition_name is not None:
            operands.append(bass2jax.partition_id_tensor())
        outs = bass2jax._bass_exec_p.bind(
            *operands,
            out_avals=tuple(out_avals),
            in_names=tuple(all_in_names),
            out_names=tuple(out_names),
            lowering_input_output_aliases=(),
            sim_require_finite=True,
            sim_require_nnan=True,
            nc=nc,
        )
        return tuple(outs)

    devices = jax.devices()[:N_CORES]
    mesh = Mesh(np.asarray(devices), ("core",))
    nspec = NamedSharding(mesh, PartitionSpec("core"))
    n_all = n_params + len(out_names)
    sharded = jax.jit(
        shard_map(_body, mesh=mesh,
                  in_specs=(PartitionSpec("core"),) * n_all,
                  out_specs=(PartitionSpec("core"),) * len(out_names),
                  check_rep=False),
        keep_unused=True)

    concat_in = [
        np.concatenate([np.asarray(in_maps[c][k]) for c in range(N_CORES)], axis=0)
        for k in in_names
    ] + [np.zeros((N_CORES * z.shape[0], *z.shape[1:]), z.dtype) for z in zero_outs]
    dev_in = [jax.device_put(a, nspec) for a in concat_in]

    for _ in range(warmup):
        outs = sharded(*dev_in)
    jax.block_until_ready(outs)

    t0 = time.perf_counter()
    for _ in range(iters):
        outs = sharded(*dev_in)
    jax.block_until_ready(outs)
    t1 = time.perf_counter()
    per_call = (t1 - t0) / iters

    return {"pipelined_ns": per_call * 1e9}


if __name__ == "__main__":
    _build_nc()
    print("IR build OK")


# revision 19
# speedup vs baseline: 1.1177x; 1.0248x over previous
"""MixER MoE-hypernetwork kernel for 8 Trainium2 NeuronCores (v3).

Expert-parallel: core e handles expert e (NEXP == n_cores == 8).

v3 = v2's SBUF-resident delta + affine gathers, plus:
  - wavefront: phase-1 H-block streaming is interleaved with phase-2 layer
    emission (region order W1|W2|W3|W4), so the Silu stream starts ~15us in
    instead of after the full H load;
  - points-split: phase 2 runs twice over 1024-point halves, layer-major
    across all 16 envs, which keeps the live h-tile set at ~half SBUF cost;
  - env-pair packing: L1 runs env pairs in PE row groups (K=64 each,
    tile_position (0,0)/(64,0)), L4 in col groups (M=64 each,
    tile_position (0,0)/(0,64)) — recovers the half-idle PE array;
  - bf16 pair-packed output [8 pairs, 128, 2048], unpacked+summed on host.

delta_sb [128, 40960] bf16: partition 32*g + env, free col j*512 + c holds
H_dev col j*2048 + g*512 + c.  Free-col region layout S:
  W1 [0,4096):      S = q*256 + w,            fw1[d=g*16+q, w]
  W2 [4096,20480):  S-C2 = q*512 + kk*256+v,  fw2[p=g*32+q, kk*256+v]
  W3 [20480,36864): same as W2
  W4 [36864,40960): S-C4 = q*128 + kk*64+d,   fw4[p=g*32+q, kk*64+d]
so dw_l(env) = delta_sb[env::32, region] zipped into the [P, cols] tile.

Bias deltas come from a tiny transposed matmul with the 896 bias rows of H
(b4 duplicated into both row halves for the pair-packed L4 epilogue).
"""
import os
import numpy as np
import ml_dtypes

import concourse.bass as bass
import concourse.bacc as bacc
import concourse.tile as tile
from concourse import mybir
from concourse.bass_utils import run_bass_kernel_spmd

# ---- problem dims (hardcoded; must match the grader's setup_inputs) ----
DATA, WIDTH, CTXD, NEXP, ENVS, NPTS = 64, 256, 128, 8, 16, 2048
SIZES = [WIDTH * DATA, WIDTH, WIDTH * WIDTH, WIDTH, WIDTH * WIDTH, WIDTH,
         DATA * WIDTH, DATA]
OFFS = np.cumsum([0] + SIZES)
NET_USED = int(OFFS[-1])          # 164672

NBLK = 80
BLK = 2048
NWCOL = NBLK * 512                # 40960 free cols in delta_sb
C1, C2, C3, C4 = 0, 4096, 20480, 36864
HBPAD = 896                       # 7 chunks of 128 (b4 duplicated)
NPAIR = ENVS // 2
HNP = NPTS // 2                   # 1024-point half

F32 = mybir.dt.float32
BF16 = mybir.dt.bfloat16
BF16_NP = ml_dtypes.bfloat16

N_CORES = 8
TRACE = os.environ.get("MIXER_TRACE", "0") == "1"

if TRACE:
    # The agent image's antenv lacks axon_hooks, so run_bass_kernel_spmd's
    # trace path can't find the NTFF profile hook. Shim it with the ctypes
    # hook factory that trn_boot ships. Profiling-only; inert when TRACE=0.
    try:
        from antenv.axon_hooks import get_axon_ntff_profile_hook  # noqa: F401
    except ImportError:
        import sys as _sys
        import types as _types
        try:
            from trn_agent_boot.trn_boot import _ntff_profile_via_ctypes
            _hook = _ntff_profile_via_ctypes("/opt/axon/libaxon_pjrt.so")
            import antenv as _antenv
            _mod = _types.ModuleType("antenv.axon_hooks")
            _mod.get_axon_ntff_profile_hook = lambda: _hook
            _mod.set_axon_ntff_profile_hook = lambda h: None
            _sys.modules["antenv.axon_hooks"] = _mod
            _antenv.axon_hooks = _mod
        except Exception as _e:  # pragma: no cover - profiling is best-effort
            print(f"NTFF hook shim failed: {_e}")

LAST_RESULTS = None  # BassKernelResults of the most recent run (for test.py)

_NC_CACHE = {}
_PERM_CACHE = {}


# --------------------------------------------------------------------------
# host-side preprocessing
# --------------------------------------------------------------------------
def _build_perm():
    """perm[dev_col] = orig H row, for the 80-block weight tensor ht."""
    if "perm" in _PERM_CACHE:
        return _PERM_CACHE["perm"]
    perm = np.zeros(NBLK * BLK, dtype=np.int64)

    def dev(S, g):
        j, c = S // 512, S % 512
        return j * BLK + g * 512 + c

    # W1: dev (g, q, w): S = C1 + q*256 + w; d = g*16+q; orig = w*64 + d
    g, q, w = np.meshgrid(np.arange(4), np.arange(16), np.arange(256),
                          indexing="ij")
    perm[dev(C1 + q * 256 + w, g)] = OFFS[0] + w * DATA + g * 16 + q

    # W2/W3: dev (g, q, kk, v): S = C + q*512 + kk*256 + v;
    #        w = kk*128 + g*32 + q; orig = v*256 + w
    g, q, kk, v = np.meshgrid(np.arange(4), np.arange(32), np.arange(2),
                              np.arange(256), indexing="ij")
    wfull = kk * 128 + g * 32 + q
    perm[dev(C2 + q * 512 + kk * 256 + v, g)] = OFFS[2] + v * WIDTH + wfull
    perm[dev(C3 + q * 512 + kk * 256 + v, g)] = OFFS[4] + v * WIDTH + wfull

    # W4: dev (g, q, kk, d): S = C4 + q*128 + kk*64 + d;
    #     w = kk*128 + g*32 + q; orig = d*256 + w
    g, q, kk, d = np.meshgrid(np.arange(4), np.arange(32), np.arange(2),
                              np.arange(64), indexing="ij")
    wfull = kk * 128 + g * 32 + q
    perm[dev(C4 + q * 128 + kk * 64 + d, g)] = OFFS[6] + d * WIDTH + wfull

    bias_rows = np.concatenate([
        OFFS[1] + np.arange(WIDTH), OFFS[3] + np.arange(WIDTH),
        OFFS[5] + np.arange(WIDTH),
        OFFS[7] + np.arange(DATA), OFFS[7] + np.arange(DATA)])  # b4 twice
    _PERM_CACHE["perm"] = (perm, bias_rows)
    return perm, bias_rows


def _prep_inputs(y, ctx, W, b, H, G, beta):
    """Returns in_maps: one dict per core."""
    perm, bias_rows = _build_perm()

    # gate softmax on host (tiny)
    logits = ctx.astype(np.float32) @ G.astype(np.float32).T      # [B, E]
    m = logits.max(-1, keepdims=True)
    eg = np.exp(logits - m)
    gate = (eg / eg.sum(-1, keepdims=True)).astype(np.float32)

    # pair-stacked y: [8, 128, 2048], rows 0-63 env 2p, 64-127 env 2p+1
    yT = y.transpose(0, 2, 1)                                     # [16,64,2048]
    ytp = np.ascontiguousarray(
        yT.reshape(NPAIR, 2 * DATA, NPTS)).astype(BF16_NP)
    ctxT = np.ascontiguousarray(ctx.T).astype(BF16_NP)            # [128, 16]
    # per-group env placement skewed by 4: env e -> partition 32g + 4g + e,
    # so each env's delta rows {e+36g} hit 4 different SDMA engines
    ctxT128 = np.zeros((CTXD, 128), dtype=BF16_NP)
    for g in range(4):
        ctxT128[:, 32 * g + 4 * g:32 * g + 4 * g + ENVS] = ctxT

    in_maps = []
    for e in range(NEXP):
        be = float(beta[e])
        ib = np.float32(1.0 / be)

        scale = np.ones(NET_USED, dtype=np.float32)
        scale[OFFS[2]:OFFS[2] + WIDTH * WIDTH] = ib
        scale[OFFS[4]:OFFS[4] + WIDTH * WIDTH] = ib
        scale[OFFS[6]:OFFS[6] + DATA * WIDTH] = ib
        Hp = (H[e] * scale[:, None])[perm]                        # [163840,128]
        ht = np.ascontiguousarray(
            Hp.T.reshape(CTXD, NBLK // 2, 2 * BLK).transpose(1, 0, 2)
        ).astype(BF16_NP)

        # bias hypernet rows, transposed: [128 ctx, 896]
        bscale = np.concatenate([np.full(768, be, np.float32),
                                 np.ones(128, np.float32)])
        hb = np.ascontiguousarray(
            (H[e][bias_rows] * bscale[:, None]).T).astype(BF16_NP)

        # base biases per 128-chunk: [128, 7] f32 (chunk 6 = b4 twice)
        bb = np.zeros((128, 7), dtype=np.float32)
        bb[:, 0] = be * b[0][e][:128]; bb[:, 1] = be * b[0][e][128:]
        bb[:, 2] = be * b[1][e][:128]; bb[:, 3] = be * b[1][e][128:]
        bb[:, 4] = be * b[2][e][:128]; bb[:, 5] = be * b[2][e][128:]
        bb[:64, 6] = b[3][e]; bb[64:, 6] = b[3][e]

        w1t = W[0][e].T.astype(np.float32)                        # [64, 256]
        w1tp = np.ascontiguousarray(
            np.concatenate([w1t, w1t], axis=0)).astype(BF16_NP)   # [128, 256]
        w2t = np.ascontiguousarray(
            (W[1][e].T * ib).reshape(2, 128, WIDTH).transpose(1, 0, 2)
            .reshape(128, 2 * WIDTH)).astype(BF16_NP)             # [128, 512]
        w3t = np.ascontiguousarray(
            (W[2][e].T * ib).reshape(2, 128, WIDTH).transpose(1, 0, 2)
            .reshape(128, 2 * WIDTH)).astype(BF16_NP)
        w4t = (W[3][e].T * ib).reshape(2, 128, DATA).transpose(1, 0, 2) \
            .reshape(128, 2 * DATA)
        w4tp = np.ascontiguousarray(
            np.concatenate([w4t, w4t], axis=1)).astype(BF16_NP)   # [128, 256]

        # per-pair gate, row-half-selected: gatep[p, pr] = gate[2pr + (p>=64)]
        gatep = np.zeros((128, NPAIR), dtype=np.float32)
        gatep[:64, :] = gate[::2, e]
        gatep[64:, :] = gate[1::2, e]

        in_maps.append({
            "ht": ht, "hb": hb, "bb": bb, "ctxt": ctxT128, "ytp": ytp,
            "w1tp": w1tp, "w2t": w2t, "w3t": w3t, "w4tp": w4tp,
            "gatep": gatep,
            "beta": np.array([be], dtype=np.float32),
        })
    return in_maps


# --------------------------------------------------------------------------
# device kernel (SPMD program, one expert per core)
# --------------------------------------------------------------------------
def _build_nc():
    if "nc" in _NC_CACHE:
        return _NC_CACHE["nc"]
    nc = bacc.Bacc()
    P = 128

    ht = nc.declare_dram_parameter("ht", [NBLK // 2, CTXD, 2 * BLK], BF16, isOutput=False)
    hb = nc.declare_dram_parameter("hb", [CTXD, HBPAD], BF16, isOutput=False)
    bb = nc.declare_dram_parameter("bb", [P, 7], F32, isOutput=False)
    ctxt = nc.declare_dram_parameter("ctxt", [CTXD, 128], BF16, isOutput=False)
    ytp = nc.declare_dram_parameter("ytp", [NPAIR, P, NPTS], BF16, isOutput=False)
    w1tp = nc.declare_dram_parameter("w1tp", [P, WIDTH], BF16, isOutput=False)
    w2t = nc.declare_dram_parameter("w2t", [P, 2 * WIDTH], BF16, isOutput=False)
    w3t = nc.declare_dram_parameter("w3t", [P, 2 * WIDTH], BF16, isOutput=False)
    w4tp = nc.declare_dram_parameter("w4tp", [P, 4 * DATA], BF16, isOutput=False)
    gatep = nc.declare_dram_parameter("gatep", [P, NPAIR], F32, isOutput=False)
    beta = nc.declare_dram_parameter("beta", [1], F32, isOutput=False)
    out = nc.declare_dram_parameter("out", [NPAIR, P, NPTS], BF16, isOutput=True)

    SILU = mybir.ActivationFunctionType.Silu
    MULT, ADD = mybir.AluOpType.mult, mybir.AluOpType.add

    def _bcast(handle, parts):
        ap = handle[:]
        return bass.AP(tensor=ap.tensor, offset=ap.offset,
                       ap=[[0, parts]] + list(ap.ap))

    with tile.TileContext(nc) as tc:
        with tc.tile_pool(name="const", bufs=1) as const, \
             tc.tile_pool(name="delta", bufs=1) as dpool, \
             tc.tile_pool(name="p1ps", bufs=2, space="PSUM") as p1ps, \
             tc.tile_pool(name="htp", bufs=3) as htp, \
             tc.tile_pool(name="fw", bufs=2) as fwp, \
             tc.tile_pool(name="fw3", bufs=3) as fwp3, \
             tc.tile_pool(name="ypool", bufs=2) as ypool, \
             tc.tile_pool(name="hpool", bufs=36) as hpool, \
             tc.tile_pool(name="opool", bufs=3) as opool, \
             tc.tile_pool(name="psp", bufs=3, space="PSUM") as psp:

            delta_sb = dpool.tile([P, NWCOL], BF16)

            # ---- minimal consts needed by phase 1 ----
            ctx_sb = const.tile([CTXD, 128], BF16)
            nc.sync.dma_start(out=ctx_sb, in_=ctxt[:, :])
            beta_sb = const.tile([P, 1], F32)
            nc.sync.dma_start(out=beta_sb, in_=_bcast(beta, P))
            gatep_sb = const.tile([P, NPAIR], F32)
            hb_sb = const.tile([CTXD, HBPAD], BF16)
            bb_sb = const.tile([P, 7], F32)
            w1tp_sb = const.tile([P, WIDTH], BF16)
            w2t_sb = const.tile([P, 2 * WIDTH], BF16)
            w3t_sb = const.tile([P, 2 * WIDTH], BF16)
            w4tp_sb = const.tile([P, 4 * DATA], BF16)
            fb_all = const.tile([P, 7 * ENVS], F32)
            fb4gp = const.tile([P, NPAIR], F32)

            def emit_consts_and_bias():
                nc.sync.dma_start(out=gatep_sb, in_=gatep[:, :])
                nc.sync.dma_start(out=hb_sb, in_=hb[:, :])
                nc.sync.dma_start(out=bb_sb, in_=bb[:, :])
                nc.sync.dma_start(out=w1tp_sb, in_=w1tp[:, :])
                nc.sync.dma_start(out=w2t_sb, in_=w2t[:, :])
                nc.sync.dma_start(out=w3t_sb, in_=w3t[:, :])
                nc.sync.dma_start(out=w4tp_sb, in_=w4tp[:, :])
                for k in range(7):
                    psb = p1ps.tile([P, 512], F32, tag="p1", name=f"psb_{k}")
                    nc.tensor.matmul(
                        psb[:, 0:ENVS],
                        lhsT=hb_sb[:, k * 128:(k + 1) * 128],
                        rhs=ctx_sb[:, 0:ENVS], start=True, stop=True)
                    nc.vector.tensor_scalar(
                        out=fb_all[:, k * ENVS:(k + 1) * ENVS],
                        in0=psb[:, 0:ENVS],
                        scalar1=bb_sb[:, k:k + 1], scalar2=None, op0=ADD)
                nc.vector.tensor_mul(
                    out=fb4gp[0:DATA, :],
                    in0=fb_all[0:DATA, 6 * ENVS + 0:7 * ENVS:2],
                    in1=gatep_sb[0:DATA, :])
                nc.vector.tensor_mul(
                    out=fb4gp[DATA:P, :],
                    in0=fb_all[DATA:P, 6 * ENVS + 1:7 * ENVS:2],
                    in1=gatep_sb[DATA:P, :])

            # ---- emission helpers ----
            def emit_block(jj):
                # one 1MB DMA covering blocks 2*jj and 2*jj+1
                htt = htp.tile([CTXD, 2 * BLK], BF16, tag="ht", name=f"ht_{jj}")
                nc.sync.dma_start(out=htt, in_=ht[jj])
                for sub in range(2):
                    j = 2 * jj + sub
                    ps = p1ps.tile([P, 512], F32, tag="p1", name=f"ps1_{j}")
                    for g in range(4):
                        nc.tensor.matmul(
                            ps[32 * g:32 * (g + 1), :],
                            lhsT=ctx_sb[:, 32 * g:32 * (g + 1)],
                            rhs=htt[:, sub * BLK + g * 512:
                                    sub * BLK + (g + 1) * 512],
                            start=True, stop=True,
                            tile_position=(0, 32 * g))
                    nc.vector.tensor_copy(
                        out=delta_sb[:, j * 512:(j + 1) * 512], in_=ps)

            hmap = {}   # (pr, half) -> {env_slot: [h_mt0, h_mt1]}
            fw1map, fw23map, fw4map, ymap = {}, {}, {}, {}

            def prep_L1(pr, half):
                eA, eB = 2 * pr, 2 * pr + 1
                dw1 = fwp.tile([P, WIDTH], BF16, tag="dw1",
                               name=f"dw1_{pr}_{half}")
                nc.gpsimd.dma_start(out=dw1[0:DATA, :],
                                    in_=delta_sb[eA::36, C1:C2])
                nc.gpsimd.dma_start(out=dw1[DATA:P, :],
                                    in_=delta_sb[eB::36, C1:C2])
                f1 = fwp.tile([P, WIDTH], BF16, tag="fw1",
                              name=f"fw1_{pr}_{half}")
                nc.vector.tensor_add(out=f1, in0=w1tp_sb, in1=dw1)
                yst = ypool.tile([P, HNP], BF16, tag="y",
                                 name=f"y_{pr}_{half}")
                nc.scalar.dma_start(
                    out=yst, in_=ytp[pr][:, half * HNP:(half + 1) * HNP])
                fw1map[(pr, half)] = f1
                ymap[(pr, half)] = yst

            def mm_L1(pr, half):
                eA, eB = 2 * pr, 2 * pr + 1
                f1 = fw1map.pop((pr, half))
                yst = ymap.pop((pr, half))
                hA, hB = [], []
                for mt in range(2):
                    psA = psp.tile([P, HNP], F32, tag="ps",
                                   name=f"psA1_{pr}_{half}_{mt}")
                    psB = psp.tile([P, HNP], F32, tag="ps",
                                   name=f"psB1_{pr}_{half}_{mt}")
                    for t in range(2):
                        nc.tensor.matmul(
                            psA[:, t * 512:(t + 1) * 512],
                            lhsT=f1[0:DATA, mt * P:(mt + 1) * P],
                            rhs=yst[0:DATA, t * 512:(t + 1) * 512],
                            start=True, stop=True, tile_position=(0, 0))
                        nc.tensor.matmul(
                            psB[:, t * 512:(t + 1) * 512],
                            lhsT=f1[DATA:P, mt * P:(mt + 1) * P],
                            rhs=yst[DATA:P, t * 512:(t + 1) * 512],
                            start=True, stop=True, tile_position=(64, 0))
                    ha = hpool.tile([P, HNP], BF16, tag="h",
                                    name=f"hA1_{pr}_{half}_{mt}")
                    nc.scalar.activation(
                        out=ha, in_=psA[:, :], func=SILU,
                        bias=fb_all[:, mt * ENVS + eA:mt * ENVS + eA + 1],
                        scale=beta_sb[:, 0:1])
                    hb_ = hpool.tile([P, HNP], BF16, tag="h",
                                     name=f"hB1_{pr}_{half}_{mt}")
                    nc.scalar.activation(
                        out=hb_, in_=psB[:, :], func=SILU,
                        bias=fb_all[:, mt * ENVS + eB:mt * ENVS + eB + 1],
                        scale=beta_sb[:, 0:1])
                    hA.append(ha)
                    hB.append(hb_)
                hmap[(pr, half)] = {0: hA, 1: hB}

            def prep_L23(pr, half, li):
                base, wt = ((C2, w2t_sb), (C3, w3t_sb))[li]
                fls = []
                for slot in range(2):
                    env = 2 * pr + slot
                    dw = fwp3.tile([P, 2 * WIDTH], BF16, tag=f"dw2{slot}",
                                  name=f"dw{li}_{pr}_{half}_{slot}")
                    nc.gpsimd.dma_start(out=dw,
                                        in_=delta_sb[env::36, base:base + 16384])
                    fl = fwp3.tile([P, 2 * WIDTH], BF16, tag=f"fw2{slot}",
                                  name=f"fwl{li}_{pr}_{half}_{slot}")
                    nc.vector.tensor_add(out=fl, in0=wt, in1=dw)
                    fls.append(fl)
                fw23map[(pr, half, li)] = fls

            def mm_L23(pr, half, li):
                hprev = hmap[(pr, half)]
                fls = fw23map.pop((pr, half, li))
                hcur = {}
                for slot in range(2):
                    env = 2 * pr + slot
                    fl = fls[slot]
                    hs = []
                    for mm in range(2):
                        ps = psp.tile([P, HNP], F32, tag="ps",
                                      name=f"ps{li}_{pr}_{half}_{slot}_{mm}")
                        for kk in range(2):
                            for t in range(2):
                                nc.tensor.matmul(
                                    ps[:, t * 512:(t + 1) * 512],
                                    lhsT=fl[:, kk * WIDTH + mm * P:
                                            kk * WIDTH + (mm + 1) * P],
                                    rhs=hprev[slot][kk][:, t * 512:(t + 1) * 512],
                                    start=(kk == 0), stop=(kk == 1))
                        hn = hpool.tile([P, HNP], BF16, tag="h",
                                        name=f"h{li}_{pr}_{half}_{slot}_{mm}")
                        nc.scalar.activation(
                            out=hn, in_=ps[:, :], func=SILU,
                            bias=fb_all[:, (2 + 2 * li + mm) * ENVS + env:
                                        (2 + 2 * li + mm) * ENVS + env + 1],
                            scale=beta_sb[:, 0:1])
                        hs.append(hn)
                    hcur[slot] = hs
                hmap[(pr, half)] = hcur

            def prep_L4(pr, half):
                eA, eB = 2 * pr, 2 * pr + 1
                dw4 = fwp.tile([P, 4 * DATA], BF16, tag="dw4",
                               name=f"dw4_{pr}_{half}")
                nc.gpsimd.dma_start(out=dw4[:, 0:2 * DATA],
                                    in_=delta_sb[eA::36, C4:NWCOL])
                nc.gpsimd.dma_start(out=dw4[:, 2 * DATA:4 * DATA],
                                    in_=delta_sb[eB::36, C4:NWCOL])
                f4 = fwp.tile([P, 4 * DATA], BF16, tag="fw4",
                              name=f"fw4_{pr}_{half}")
                nc.vector.tensor_add(out=f4, in0=w4tp_sb, in1=dw4)
                fw4map[(pr, half)] = f4

            def mm_L4(pr, half):
                hprev = hmap.pop((pr, half))
                f4 = fw4map.pop((pr, half))
                ps4 = psp.tile([P, HNP], F32, tag="ps",
                               name=f"ps4_{pr}_{half}")
                for kk in range(2):
                    for t in range(2):
                        nc.tensor.matmul(
                            ps4[0:DATA, t * 512:(t + 1) * 512],
                            lhsT=f4[:, kk * DATA:(kk + 1) * DATA],
                            rhs=hprev[0][kk][:, t * 512:(t + 1) * 512],
                            start=(kk == 0), stop=(kk == 1),
                            tile_position=(0, 0))
                        nc.tensor.matmul(
                            ps4[DATA:P, t * 512:(t + 1) * 512],
                            lhsT=f4[:, 2 * DATA + kk * DATA:
                                    2 * DATA + (kk + 1) * DATA],
                            rhs=hprev[1][kk][:, t * 512:(t + 1) * 512],
                            start=(kk == 0), stop=(kk == 1),
                            tile_position=(0, 64))
                osb = opool.tile([P, HNP], BF16, tag="o",
                                 name=f"o_{pr}_{half}")
                nc.vector.tensor_scalar(
                    out=osb, in0=ps4[:, :],
                    scalar1=gatep_sb[:, pr:pr + 1],
                    scalar2=fb4gp[:, pr:pr + 1],
                    op0=MULT, op1=ADD)
                nc.sync.dma_start(
                    out=out[pr][:, half * HNP:(half + 1) * HNP], in_=osb)

            # ---- wavefront schedule (super-blocks of 2) ----
            emit_block(0)
            emit_block(1)
            emit_consts_and_bias()
            emit_block(2)
            emit_block(3)
            for pr in range(NPAIR):                     # L1 h0 | W2 stream
                prep_L1(pr, 0)
                mm_L1(pr, 0)
                emit_block(4 + 2 * pr)
                emit_block(5 + 2 * pr)
            for pr in range(NPAIR):                     # L2 h0 | W3 stream
                prep_L23(pr, 0, 0)
                mm_L23(pr, 0, 0)
                emit_block(20 + 2 * pr)
                emit_block(21 + 2 * pr)
            for pr in range(NPAIR):                     # L3 h0 | W4 stream
                prep_L23(pr, 0, 1)
                mm_L23(pr, 0, 1)
                if pr < 4:
                    emit_block(36 + pr)
            for pr in range(NPAIR):                     # seam: keep ACT fed
                prep_L1(pr, 1)
                mm_L1(pr, 1)
                prep_L4(pr, 0)
                mm_L4(pr, 0)
            for pr in range(NPAIR):
                prep_L23(pr, 1, 0)
                mm_L23(pr, 1, 0)
            for pr in range(NPAIR):                     # tail: L4 interleaved
                prep_L23(pr, 1, 1)
                mm_L23(pr, 1, 1)
                if pr >= 1:
                    prep_L4(pr - 1, 1)
                    mm_L4(pr - 1, 1)
            prep_L4(NPAIR - 1, 1)
            mm_L4(NPAIR - 1, 1)

    nc.compile()
    _NC_CACHE["nc"] = nc
    return nc


# --------------------------------------------------------------------------
# entry point
# --------------------------------------------------------------------------
def kernel(t, y, ctx, W1, b1, W2, b2, W3, b3, W4, b4, H, G, beta):
    global LAST_RESULTS
    y = np.asarray(y, np.float32)
    ctx = np.asarray(ctx, np.float32)
    H = np.asarray(H, np.float32)
    G = np.asarray(G, np.float32)
    beta = np.asarray(beta, np.float32)
    W = [np.asarray(w, np.float32) for w in (W1, W2, W3, W4)]
    b = [np.asarray(x, np.float32) for x in (b1, b2, b3, b4)]

    in_maps = _prep_inputs(y, ctx, W, b, H, G, beta)
    nc = _build_nc()
    res = run_bass_kernel_spmd(
        nc, in_maps, list(range(N_CORES)),
        trace=TRACE, trace_cores=None)
    LAST_RESULTS = res

    total = np.zeros((ENVS, DATA, NPTS), np.float32)
    for e in range(N_CORES):
        o = res.results[e]["out"].astype(np.float32)   # [8, 128, 2048]
        total[0::2] += o[:, :DATA, :]
        total[1::2] += o[:, DATA:, :]
    return np.ascontiguousarray(total.transpose(0, 2, 1))


def measure_exec_ns(inputs, iters=64, warmup=4):
    """Steady-state per-execution time of the compiled NEFF on 8 cores.

    Keeps inputs device-resident and measures the marginal wall time of
    pipelined executions. The result still contains per-call dispatch
    overhead (compare against a trivial kernel's floor for the difference).
    Used by test.py only; the grading path never calls this.
    """
    import time
    import jax
    from jax.sharding import Mesh, PartitionSpec, NamedSharding
    from jax.experimental.shard_map import shard_map
    from concourse import bass2jax, mybir as _mybir

    y = np.asarray(inputs["y"], np.float32)
    ctx = np.asarray(inputs["ctx"], np.float32)
    H = np.asarray(inputs["H"], np.float32)
    G = np.asarray(inputs["G"], np.float32)
    beta = np.asarray(inputs["beta"], np.float32)
    W = [np.asarray(inputs[k], np.float32) for k in ("W1", "W2", "W3", "W4")]
    b = [np.asarray(inputs[k], np.float32) for k in ("b1", "b2", "b3", "b4")]
    in_maps = _prep_inputs(y, ctx, W, b, H, G, beta)
    nc = _build_nc()

    bass2jax.install_neuronx_cc_hook()
    partition_name = nc.partition_id_tensor.name if nc.partition_id_tensor else None
    in_names, out_names, out_avals, zero_outs = [], [], [], []
    for alloc in nc.m.functions[0].allocations:
        if not isinstance(alloc, _mybir.MemoryLocationSet):
            continue
        name = alloc.memorylocations[0].name
        if alloc.kind == "ExternalInput":
            if name != partition_name:
                in_names.append(name)
        elif alloc.kind == "ExternalOutput":
            shape = tuple(alloc.tensor_shape)
            dtype = _mybir.dt.np(alloc.dtype)
            out_names.append(name)
            out_avals.append(jax.core.ShapedArray(shape, dtype))
            zero_outs.append(np.zeros(shape, dtype))
    n_params = len(in_names)
    all_in_names = in_names + out_names
    if partition_name is not None:
        all_in_names.append(partition_name)

    def _body(*args):
        operands = list(args)
        if partition_name is not None:
            operands.append(bass2jax.partition_id_tensor())
        outs = bass2jax._bass_exec_p.bind(
            *operands,
            out_avals=tuple(out_avals),
            in_names=tuple(all_in_names),
            out_names=tuple(out_names),
            lowering_input_output_aliases=(),
            sim_require_finite=True,
            sim_require_nnan=True,
            nc=nc,
        )
        return tuple(outs)

    devices = jax.devices()[:N_CORES]
    mesh = Mesh(np.asarray(devices), ("core",))
    nspec = NamedSharding(mesh, PartitionSpec("core"))
    n_all = n_params + len(out_names)
    sharded = jax.jit(
        shard_map(_body, mesh=mesh,
                  in_specs=(PartitionSpec("core"),) * n_all,
                  out_specs=(PartitionSpec("core"),) * len(out_names),
                  check_rep=False),
        keep_unused=True)

    concat_in = [
        np.concatenate([np.asarray(in_maps[c][k]) for c in range(N_CORES)], axis=0)
        for k in in_names
    ] + [np.zeros((N_CORES * z.shape[0], *z.shape[1:]), z.dtype) for z in zero_outs]
    dev_in = [jax.device_put(a, nspec) for a in concat_in]

    for _ in range(warmup):
        outs = sharded(*dev_in)
    jax.block_until_ready(outs)

    t0 = time.perf_counter()
    for _ in range(iters):
        outs = sharded(*dev_in)
    jax.block_until_ready(outs)
    t1 = time.perf_counter()
    per_call = (t1 - t0) / iters

    return {"pipelined_ns": per_call * 1e9}


if __name__ == "__main__":
    _build_nc()
    print("IR build OK")
